# revision 60
# baseline (speedup 1.0000x reference)
"""Trainium2 Bass kernel for GQA MultiHeadAttention (B=1, S=2048, D=4096,
H=32 query heads, HKV=8 kv heads, DK=DV=128), tensor-parallel across heads
on 8 NeuronCores.

Sharding: core c owns query heads 4c..4c+3 and kv head c (GQA group) and
computes its 512 attention features. The transposed attention output is
AllGathered across cores in four per-q-block collectives (overlapped with
attention compute of later blocks), then each core computes a 512-row slice
of the transposed output projection. Host side: pre-transpose/cast inputs,
final concat + transpose.

Schedule:
- P1 interleaves the K, Q AND V projections in one pass over the kT/qT/vT
  chunk streams (DMA ~50 MB ~= PE work), so no projection phase is
  DMA-starved. K holds 4 PSUM banks for the full 32-chunk accumulation;
  Q and V accumulate per-8-chunk-block partials in the two 2-bank pools
  (sb01 in psAD, sb23 in psO, flushed sb-major so the flush overlaps the
  other half's matmuls) and add into bf16 SBUF accumulators in place.
- Then: K evac, PE transpose of VT (batched), and attention. Scores for a
  4-ktile quad go through a 2-deep ring of [128,2,512] PSUM pair-tiles
  (exp of pair g overlaps scores of pair g+1); exp is batched per pair
  ([128,1024] per activation). PV/den matmuls skip fully-masked,
  pair-aligned column ranges on the diagonal quads.
- Emission order attn0,g0,attn1,g1,attn2,g2,op0,attn3,g3,op1,op2,op3 keeps
  the in-order PE queue from ever parking on a collective-dependent
  out-proj matmul while attention work is still available.

Self-contained: hardcodes all shapes; inputs are the full unsharded tensors
keyed as in the problem's setup_inputs().
"""

import numpy as np
import ml_dtypes

import concourse.bacc as bacc
import concourse.mybir as mybir
from concourse.tile import TileContext
from concourse.bass_utils import run_bass_kernel_spmd

BF16 = mybir.dt.bfloat16
F32 = mybir.dt.float32

N_CORES = 8
S = 2048            # sequence length
D = 4096            # model dim
DK = 128            # head dim
NH_LOC = 4          # query heads per core
FLOC = NH_LOC * DK  # per-core attention features (512)
NDC = D // 128      # contraction chunks of 128 over D (32)
SB = 512            # q/s block width
NSB = S // SB       # 4
NST = S // 128      # 16 seq tiles of 128
NBLK = 4            # dc blocks for Q/V accumulation
BLK = NDC // NBLK   # 8 chunks per block
NE = 4              # E ring depth (4-ktile quads)

_DMA_TYPES = ("InstDMACopy", "InstDMATranspose")


def _legalize_dma_waits(nc):
    """DMA pseudo-instructions encode at most ONE sem wait (the ISA events
    slot). If Tile's sem assignment leaves more on a DMA, walrus rejects it
    ("Too many sync wait commands"). Hoist all but the last wait onto fresh
    nop instructions inserted immediately before the DMA on the same engine —
    the sequencer executes them in order, so semantics are identical."""
    ctr = 0
    for f in nc.m.functions:
        for blk in f.blocks:
            out = []
            changed = False
            for inst in blk.instructions:
                si = inst.sync_info
                if (
                    si is not None
                    and len(si.on_wait) > 1
                    and type(inst).__name__ in _DMA_TYPES
                ):
                    waits = list(si.on_wait)
                    for w in waits[:-1]:
                        nop = mybir.InstNoOp(
                            name=f"I-dmawaitfix-{ctr}", ins=[], outs=[]
                        )
                        ctr += 1
                        nop.engine = inst.engine
                        nop.sync_info = mybir.SyncInfo(on_wait=[w], on_update=[])
                        out.append(nop)
                    inst.sync_info = mybir.SyncInfo(
                        on_wait=[waits[-1]], on_update=list(si.on_update)
                    )
                    changed = True
                out.append(inst)
            if changed:
                blk.instructions = out
    return ctr


def _build(stage=4, nrep=1):
    # stage: 1=projections only, 2=+attention (no collective), 4=full
    nc = bacc.Bacc("TRN2", target_bir_lowering=False, num_devices=N_CORES,
                   dynamic_dma_scratch_size=2048)

    # ---- I/O ----
    qT = nc.dram_tensor("qT", [D, S], BF16, kind="ExternalInput")
    kT = nc.dram_tensor("kT", [D, S], BF16, kind="ExternalInput")
    vT = nc.dram_tensor("vT", [D, S], BF16, kind="ExternalInput")
    wqT = nc.dram_tensor("wqT", [128, NDC, FLOC], BF16, kind="ExternalInput")
    wkT = nc.dram_tensor("wkT", [128, NDC, DK], BF16, kind="ExternalInput")
    wvT = nc.dram_tensor("wvT", [128, NDC, DK], BF16, kind="ExternalInput")
    wdT = nc.dram_tensor("wdT", [128, NDC, FLOC], BF16, kind="ExternalInput")
    masks = nc.dram_tensor("masks", [128, 4 * SB], BF16, kind="ExternalInput")
    ident = nc.dram_tensor("ident", [128, 128], BF16, kind="ExternalInput")
    outT = nc.dram_tensor("outT", [FLOC, S], BF16, kind="ExternalOutput")

    with TileContext(nc) as tc:
        with (
            tc.tile_pool(name="consts", bufs=1) as consts,
            tc.tile_pool(name="bigw", bufs=1) as bigw,
            tc.tile_pool(name="persist", bufs=1) as persist,
            tc.tile_pool(name="qstream", bufs=9) as qstream,
            tc.tile_pool(name="kstream", bufs=6) as kstream,
            tc.tile_pool(name="vstream", bufs=8) as vstream,
            tc.tile_pool(name="small", bufs=3) as small,
            tc.tile_pool(name="attnout", bufs=3) as attnout,
            tc.tile_pool(name="atin", bufs=3) as atin,
            tc.tile_pool(name="osb", bufs=2) as osb,
            # PSUM: 8 banks.
            #   P1: K-hold (psS 2x[128,2,512], 4 banks) + Q/V half-pass
            #       rotation (psAD 2 + psO 2)
            #   A+O: scores pair ring (psS, 4) + att/den (psAD, 2) +
            #        outproj (psO, 2)
            tc.tile_pool(name="psS", bufs=2, space="PSUM") as psS,
            tc.tile_pool(name="psAD", bufs=1, space="PSUM") as psAD,
            tc.tile_pool(name="psO", bufs=1, space="PSUM") as psO,
            tc.tile_pool(name="dram", bufs=1, space="DRAM") as dram,
        ):
            def one_rep(rep):
                # ---- small constants ----
                wk_sb = consts.tile([128, NDC, DK], BF16, name="wk_sb")
                wk_r = wkT
                nc.sync.dma_start(wk_sb[:, 0:BLK, :], wk_r[:, 0:BLK, :])
                ones_sb = consts.tile([128, 128], BF16, name="ones_sb")
                nc.vector.memset(ones_sb[:], 1.0)
                ident_sb = consts.tile([128, 128], BF16, name="ident_sb")
                wv_sb = consts.tile([128, NDC, DK], BF16, name="wv_sb")
                wv_r = wvT

                # persistent activations (QT/VT accumulated in place, bf16)
                QT_sb = persist.tile([128, NH_LOC, NSB, SB], BF16,
                                     name="QT_sb")
                KT_sb = persist.tile([128, S], BF16, name="KT_sb")
                VT_sb = persist.tile([128, NSB, SB], BF16, name="VT_sb")
                V_sb = persist.tile([128, NST, DK], BF16, name="V_sb")

                # wq shares the bigw slot with wd (wd DMA ordered after wq's
                # last read by the tile ring)
                wq_sb = bigw.tile([128, NDC, FLOC], BF16, name="wq_sb", tag="bigw")
                wq_r = wqT

                # per-q-block DRAM bounce buffers for the collectives
                attn_loc = [
                    dram.tile([FLOC, SB], BF16, name=f"attn_loc{qb}", tag=f"al{qb}")
                    for qb in range(NSB)
                ]
                attn_gath = [
                    dram.tile([N_CORES * FLOC, SB], BF16, name=f"attn_gath{qb}",
                              tag=f"ag{qb}", addr_space="Shared")
                    for qb in range(NSB)
                ]

                # ================= P1: K + Q + V projections, one interleaved
                # ================= pass over the kT/qT/vT chunk streams
                k_ps = [psS.tile([128, 2, SB], F32, name=f"k_ps{i}", tag="psS")
                        for i in range(2)]  # sb01, sb23 held all of P1

                def proj_half(ps_pool, tag, w_sb, chunks, dcb, f, sbs, dst_row):
                    """8-chunk accumulation of 2 sb-blocks into a 2-bank tile,
                    then flush-add into the bf16 accumulator row."""
                    ps = ps_pool.tile([128, 2, SB], F32, name=f"{tag}", tag=tag)
                    for sb in sbs:
                        for i in range(BLK):
                            nc.tensor.matmul(
                                ps[:, sb % 2, :],
                                lhsT=w_sb[:, dcb * BLK + i,
                                          f * 128:(f + 1) * 128]
                                if w_sb is wq_sb else
                                w_sb[:, dcb * BLK + i, :],
                                rhs=chunks[i][:, sb * SB:(sb + 1) * SB],
                                start=(i == 0),
                                stop=(i == BLK - 1),
                            )
                    dst = dst_row[:, sbs[0]:sbs[1] + 1, :]
                    if dcb == 0:
                        nc.vector.tensor_copy(dst, ps[:, :, :])
                    else:
                        nc.vector.tensor_tensor(
                            dst, dst, ps[:, :, :], mybir.AluOpType.add)

                for dcb in range(NBLK):
                    q_chunks = []
                    if dcb == 0:
                        # startup block: pair kt/qt DMAs and interleave the K
                        # matmuls with Qf0's accumulation so the PE isn't
                        # paced by a single stream
                        q_ps_a = psAD.tile([128, 2, SB], F32, name="psAD",
                                           tag="psAD")
                        q_ps_b = psO.tile([128, 2, SB], F32, name="psO",
                                          tag="psO")
                        for i in range(BLK):
                            kt_c = kstream.tile([128, S], BF16, name="kt_c",
                                                tag="kt")
                            nc.sync.dma_start(kt_c[:], kT[i * 128:(i + 1) * 128, :])
                            qt_c = qstream.tile([128, S], BF16, name="qt_c",
                                                tag="qt")
                            nc.sync.dma_start(qt_c[:], qT[i * 128:(i + 1) * 128, :])
                            if i == 0:
                                nc.sync.dma_start(
                                    wq_sb[:, 0:1, :], wq_r[:, 0:1, :])
                            if i == 1:
                                nc.sync.dma_start(
                                    wq_sb[:, 1:BLK, :], wq_r[:, 1:BLK, :])
                            q_chunks.append(qt_c)
                            if i == 4:
                                nc.sync.dma_start(
                                    wk_sb[:, BLK:2 * BLK, :],
                                    wk_r[:, BLK:2 * BLK, :])
                            if i == 6:
                                nc.sync.dma_start(
                                    wv_sb[:, 0:BLK, :], wv_r[:, 0:BLK, :])
                            for sb in range(NSB):
                                nc.tensor.matmul(
                                    k_ps[sb // 2][:, sb % 2, :],
                                    lhsT=wk_sb[:, i, :],
                                    rhs=kt_c[:, sb * SB:(sb + 1) * SB],
                                    start=(i == 0),
                                    stop=False,
                                )
                            for sb in range(NSB):
                                nc.tensor.matmul(
                                    (q_ps_a if sb < 2 else q_ps_b)[:, sb % 2, :],
                                    lhsT=wq_sb[:, i, 0:128],
                                    rhs=qt_c[:, sb * SB:(sb + 1) * SB],
                                    start=(i == 0),
                                    stop=(i == BLK - 1),
                                )
                        nc.vector.tensor_copy(
                            QT_sb[:, 0, 0:2, :], q_ps_a[:, :, :])
                        nc.vector.tensor_copy(
                            QT_sb[:, 0, 2:4, :], q_ps_b[:, :, :])
                        f_list = (1, 2, 3)
                    else:
                        # K: per-chunk DMA + matmuls, 4-bank hold continues
                        for i in range(BLK):
                            dc = dcb * BLK + i
                            kt_c = kstream.tile([128, S], BF16, name="kt_c",
                                                tag="kt")
                            nc.sync.dma_start(kt_c[:], kT[dc * 128:(dc + 1) * 128, :])
                            if i == 1:  # wq piece inside the chunk stream
                                nc.sync.dma_start(
                                    wq_sb[:, dcb * BLK:(dcb + 1) * BLK, :],
                                    wq_r[:, dcb * BLK:(dcb + 1) * BLK, :])
                            if i == 4 and dcb < NBLK - 1:  # next wk piece
                                nc.sync.dma_start(
                                    wk_sb[:, (dcb + 1) * BLK:(dcb + 2) * BLK, :],
                                    wk_r[:, (dcb + 1) * BLK:(dcb + 2) * BLK, :])
                            for sb in range(NSB):
                                nc.tensor.matmul(
                                    k_ps[sb // 2][:, sb % 2, :],
                                    lhsT=wk_sb[:, dc, :],
                                    rhs=kt_c[:, sb * SB:(sb + 1) * SB],
                                    start=False,
                                    stop=(dc == NDC - 1),
                                )
                        for i in range(BLK):
                            dc = dcb * BLK + i
                            qt_c = qstream.tile([128, S], BF16, name="qt_c",
                                                tag="qt")
                            nc.sync.dma_start(qt_c[:], qT[dc * 128:(dc + 1) * 128, :])
                            q_chunks.append(qt_c)
                            if i == 1:  # wv piece
                                nc.sync.dma_start(
                                    wv_sb[:, dcb * BLK:(dcb + 1) * BLK, :],
                                    wv_r[:, dcb * BLK:(dcb + 1) * BLK, :])
                        f_list = (0, 1, 2, 3)
                    for f in f_list:
                        proj_half(psAD, "psAD", wq_sb, q_chunks, dcb, f,
                                  (0, 1), QT_sb[:, f, :, :])
                        proj_half(psO, "psO", wq_sb, q_chunks, dcb, f,
                                  (2, 3), QT_sb[:, f, :, :])
                    # V: per-chunk DMA + matmuls into a psAD+psO tile pair
                    va = psAD.tile([128, 2, SB], F32, name="vps_a", tag="psAD")
                    vb = psO.tile([128, 2, SB], F32, name="vps_b", tag="psO")
                    for i in range(BLK):
                        dc = dcb * BLK + i
                        vt_c = vstream.tile([128, S], BF16, name="vt_c", tag="vt")
                        nc.sync.dma_start(vt_c[:], vT[dc * 128:(dc + 1) * 128, :])
                        for sb in range(NSB):
                            nc.tensor.matmul(
                                (va if sb < 2 else vb)[:, sb % 2, :],
                                lhsT=wv_sb[:, dc, :],
                                rhs=vt_c[:, sb * SB:(sb + 1) * SB],
                                start=(i == 0),
                                stop=(i == BLK - 1),
                            )
                    for half, ps in enumerate((va, vb)):
                        dst = VT_sb[:, half * 2:(half + 1) * 2, :]
                        if dcb == 0:
                            nc.vector.tensor_copy(dst, ps[:, :, :])
                        else:
                            nc.vector.tensor_tensor(
                                dst, dst, ps[:, :, :], mybir.AluOpType.add)

                # K evac on the scalar engine (idle during P1) so it does
                # not queue behind the Q/V flushes on DVE
                for i in range(2):
                    nc.scalar.activation(
                        KT_sb[:, i * 2 * SB:(i + 1) * 2 * SB], k_ps[i][:, :, :],
                        mybir.ActivationFunctionType.Copy)

                masks_sb = consts.tile([128, 4, SB], BF16, name="masks_sb")
                nc.sync.dma_start(
                    masks_sb[:], masks.rearrange("p (d q) -> p d q", q=SB))
                nc.sync.dma_start(ident_sb[:], ident[:])

                # E quads: manual ring in one persistent tensor (slice deps
                # tracked by Tile); [slot][kt_mod4][q]
                E_sb = persist.tile([128, NE, 4, SB], BF16, name="E_sb")
                nc.vector.memset(E_sb[:], 0.0)
                e_slot = [0]

                def scores_quad(qb, h, quad):
                    """scores+exp for k-tiles 4*quad..4*quad+3 of (qb, h);
                    2-deep PSUM pair ring so exp overlaps next scores. On the
                    diagonal quad, scores/exp of tile d cover only the visible
                    q >= 128*d (the stale remainder is zeroed by the mask)."""
                    slot = e_slot[0] % NE
                    e_slot[0] += 1
                    diag = quad == qb
                    for pair in range(2):
                        sc_ps = psS.tile([128, 2, SB], F32, name="sc_ps",
                                         tag="psS")
                        for dd in range(2):
                            d = pair * 2 + dd
                            kt = 4 * quad + d
                            c0 = 128 * d if diag else 0
                            nc.tensor.matmul(
                                sc_ps[:, dd, c0:SB],
                                lhsT=KT_sb[:, kt * 128:(kt + 1) * 128],
                                rhs=QT_sb[:, h, qb, c0:SB],
                                start=True,
                                stop=True,
                            )
                        if diag:
                            for dd in range(2):
                                d = pair * 2 + dd
                                nc.scalar.activation(
                                    E_sb[:, slot, d, 128 * d:SB],
                                    sc_ps[:, dd, 128 * d:SB],
                                    mybir.ActivationFunctionType.Exp)
                        else:
                            nc.scalar.activation(
                                E_sb[:, slot, pair * 2:(pair + 1) * 2, :],
                                sc_ps[:, :, :],
                                mybir.ActivationFunctionType.Exp)
                    if diag:  # causal mask (also zeroes the stale strips)
                        nc.vector.tensor_tensor(
                            E_sb[:, slot, :, :], E_sb[:, slot, :, :],
                            masks_sb[:], mybir.AluOpType.mult)
                    return slot

                early0 = {}
                if stage >= 2:
                    # qb0 scores+exp need only KT/QT: run them while the vt
                    # tail streams and the transposes wait on VT
                    for h in range(NH_LOC):
                        early0[h] = scores_quad(0, h, 0)

                # VT[dv, s] -> V_sb[s, kt, dv], batched PE transpose
                tp = psAD.tile([128, NST, 128], BF16, name="tp", tag="psAD")
                for st in range(NST):
                    nc.tensor.transpose(
                        tp[:, st, :],
                        VT_sb[:, st // 4, (st % 4) * 128:(st % 4 + 1) * 128],
                        ident_sb[:])
                nc.vector.tensor_copy(V_sb[:], tp[:, :, :])

                if stage == 1:
                    for hh in range(NH_LOC):
                        for sb in range(NSB):
                            o_sb = osb.tile([128, SB], BF16, name="o_sb", tag="osb2")
                            nc.vector.tensor_copy(
                                o_sb[:], QT_sb[:, hh, sb, :])
                            nc.sync.dma_start(
                                outT[hh * 128:(hh + 1) * 128,
                                     sb * SB:(sb + 1) * SB], o_sb[:])
                    return

                # ================= A: attention per (qb, h) + per-qb gather,
                # ================= out-proj of earlier blocks interleaved
                wd_sb = bigw.tile([128, NDC, FLOC], BF16, name="wd_sb",
                                  tag="bigw")
                wd_r = wdT

                def attention_head(qb, h):
                    nkt = 4 * qb + 4
                    if qb == 0:
                        slots = [early0[h]]
                    else:
                        slots = [scores_quad(qb, h, quad)
                                 for quad in range(qb + 1)]
                    # alternate pools so head h+1's PV/den can start while
                    # head h's reciprocal/attn_t still read its tile
                    pool, ptag = ((psAD, "psAD") if (qb * NH_LOC + h) % 2 == 0
                                  else (psO, "psO"))
                    ad_ps = pool.tile([128, 2, SB], F32, name="ad_ps", tag=ptag)
                    att = ad_ps[:, 0, :]
                    den = ad_ps[:, 1, :]
                    def trim(kt):
                        # causal col-trim: tile kt of the diagonal quad only
                        # contributes to q >= 128*(d - d%2) (pair-aligned; the
                        # masked remainder of that range is exactly 0 in E).
                        d = kt % 4
                        return 128 * (d - d % 2) if kt // 4 == qb else 0
                    for kt in range(nkt):
                        c0 = trim(kt)
                        e_ap = E_sb[:, slots[kt // 4], kt % 4, c0:SB]
                        nc.tensor.matmul(
                            den[:, c0:SB],
                            lhsT=ones_sb[:],
                            rhs=e_ap,
                            start=(kt == 0),
                            stop=(kt == nkt - 1),
                            skip_group_check=True,
                        )
                        nc.tensor.matmul(
                            att[:, c0:SB],
                            lhsT=V_sb[:, kt, :],
                            rhs=e_ap,
                            start=(kt == 0),
                            stop=(kt == nkt - 1),
                            skip_group_check=True,
                        )
                    # attn[dv, q] /= den[q]; den rows are replicated across
                    # partitions by the all-ones lhsT.
                    rec = small.tile([128, SB], F32, name="rec", tag="rec")
                    nc.vector.reciprocal(rec[:], den)
                    attn_t = attnout.tile([128, SB], BF16, name="attn_t",
                                          tag="attn")
                    nc.vector.tensor_tensor(
                        attn_t[:], att, rec[:], mybir.AluOpType.mult)
                    nc.sync.dma_start(
                        attn_loc[qb][h * 128:(h + 1) * 128, :], attn_t[:])
                    if stage == 2:
                        o_sb = osb.tile([128, SB], BF16, name="o_sb", tag="osb2")
                        nc.vector.tensor_copy(o_sb[:], attn_t[:])
                        nc.sync.dma_start(
                            outT[h * 128:(h + 1) * 128,
                                 qb * SB:(qb + 1) * SB], o_sb[:])

                def gather(qb):
                    nc.gpsimd.collective_compute(
                        "AllGather",
                        mybir.AluOpType.bypass,
                        replica_groups=[list(range(N_CORES))],
                        ins=[attn_loc[qb][:]],
                        outs=[attn_gath[qb][:]],
                    )

                # out-proj for one q block: 2 passes of dsub pairs over the
                # gathered [4096, 512] slab (atin chunks re-read per pass).
                def outproj(qb):
                    gath_r = attn_gath[qb].rearrange("(c p) q -> p c q", p=128)
                    # one pass: dsub 0/1 accumulate in psO, dsub 2/3 in psAD
                    # (both free after attention); each atin group is read
                    # once and feeds 16 matmuls.
                    o_ps_a = psO.tile([128, 2, SB], F32, name="o_ps_a", tag="psO")
                    o_ps_b = psAD.tile([128, 2, SB], F32, name="o_ps_b", tag="psAD")
                    for g in range(NDC // 4):
                        at_c = atin.tile([128, 4, SB], BF16, name="at_c",
                                         tag="atin")
                        nc.sync.dma_start(at_c[:], gath_r[:, 4 * g:4 * g + 4, :])
                        for j in range(4):
                            fc = 4 * g + j
                            for dsub in range(4):
                                o_ps = o_ps_a if dsub < 2 else o_ps_b
                                nc.tensor.matmul(
                                    o_ps[:, dsub % 2, :],
                                    lhsT=wd_sb[:, fc,
                                               dsub * 128:(dsub + 1) * 128],
                                    rhs=at_c[:, j, :],
                                    start=(fc == 0),
                                    stop=(fc == NDC - 1),
                                )
                    for half, o_ps in enumerate((o_ps_a, o_ps_b)):
                        o_out = osb.tile([128, 2, SB], BF16, name="o_out",
                                         tag="osb")
                        if half == 0:
                            nc.vector.tensor_copy(o_out[:], o_ps[:, :, :])
                        else:  # parallel evac on the idle scalar engine
                            nc.scalar.activation(
                                o_out[:], o_ps[:, :, :],
                                mybir.ActivationFunctionType.Copy)
                        for i in range(2):
                            dsub = half * 2 + i
                            nc.sync.dma_start(
                                outT[dsub * 128:(dsub + 1) * 128,
                                     qb * SB:(qb + 1) * SB],
                                o_out[:, i, :],
                            )

                def wd_piece(j):
                    nc.sync.dma_start(
                        wd_sb[:, j * BLK:(j + 1) * BLK, :],
                        wd_r[:, j * BLK:(j + 1) * BLK, :])

                for h in range(NH_LOC):
                    attention_head(0, h)
                if stage >= 4:
                    gather(0)
                wd_piece(0)
                wd_piece(1)
                for h in range(NH_LOC):
                    attention_head(1, h)
                if stage >= 4:
                    gather(1)
                wd_piece(2)
                wd_piece(3)
                for h in range(NH_LOC):
                    attention_head(2, h)
                if stage >= 4:
                    gather(2)
                for h in range(NH_LOC):
                    attention_head(3, h)
                if stage >= 4:
                    gather(3)
                    outproj(0)
                    outproj(1)
                    outproj(2)
                    outproj(3)

            for rep in range(nrep):
                one_rep(rep)

    nc.compile()
    _legalize_dma_waits(nc)
    nc.codegen_inst_isa_subclasses()
    return nc


_NC_CACHE = {}


def _get_nc(stage=4, nrep=1):
    key = (stage, nrep)
    if key not in _NC_CACHE:
        _NC_CACHE[key] = _build(stage, nrep)
    return _NC_CACHE[key]


def _make_in_maps(q, k, v, Wq, Wk, Wv, Wd):
    bf = ml_dtypes.bfloat16
    scale = np.float32(DK) ** -0.5
    qT = np.ascontiguousarray(q.reshape(S, D).T).astype(bf)
    kT = np.ascontiguousarray(k.reshape(S, D).T).astype(bf)
    vT = np.ascontiguousarray(v.reshape(S, D).T).astype(bf)

    kp = np.arange(128, dtype=np.int32)[:, None]
    qf = np.arange(SB, dtype=np.int32)[None, :]
    masks = np.concatenate(
        [(qf >= kp + 128 * d).astype(np.float32) for d in range(4)], axis=1
    ).astype(bf)
    ident = np.eye(128, dtype=np.float32).astype(bf)

    in_maps = []
    for c in range(N_CORES):
        fs = slice(FLOC * c, FLOC * (c + 1))
        ks = slice(DK * c, DK * (c + 1))
        def chunked(wT):  # [D, X] -> [128, NDC, X] (partition-major chunks)
            return np.ascontiguousarray(
                wT.reshape(NDC, 128, wT.shape[1]).swapaxes(0, 1)).astype(bf)

        in_maps.append({
            "qT": qT,
            "kT": kT,
            "vT": vT,
            "wqT": chunked((Wq[fs, :] * scale).T),
            "wkT": chunked(Wk[ks, :].T),
            "wvT": chunked(Wv[ks, :].T),
            "wdT": chunked(Wd[fs, :].T),
            "masks": masks,
            "ident": ident,
        })
    return in_maps


def _assemble(results):
    outT_full = np.concatenate(
        [np.asarray(r["outT"], dtype=np.float32) for r in results], axis=0)
    return np.ascontiguousarray(outT_full.T).reshape(1, S, D)


def kernel(q, k, v, Wq, Wk, Wv, Wd, _trace=False, **_ignored):
    nc = _get_nc()
    in_maps = _make_in_maps(
        np.asarray(q, np.float32), np.asarray(k, np.float32),
        np.asarray(v, np.float32), np.asarray(Wq, np.float32),
        np.asarray(Wk, np.float32), np.asarray(Wv, np.float32),
        np.asarray(Wd, np.float32),
    )
    res = run_bass_kernel_spmd(
        nc, in_maps, core_ids=list(range(N_CORES)), trace=_trace
    )
    out = _assemble(res.results)
    if _trace:
        return out, res
    return out
